# revision 1
# baseline (speedup 1.0000x reference)
"""CPC loss (nn_CPCLossV2) Trainium2 kernel.

Strategy (data-parallel over the n=4096 group axis, 512 groups/core x 8 cores):
  - Host: pure layout prep (transposes/slices of inputs, gather-index
    arithmetic + int16 wrapping). No reference math on host beyond the
    final mean of 8 per-core partial sums.
  - Device, per core:
      * cast the f32 embedding table to an internal fp16 copy (SWDGE
        cast-DMA, HBM->HBM)
      * predicts = hist_x @ W.T + b via PE (fp32, both orientations:
        [h, g] for scalar operands and [g, h] for the positive logit)
      * for each batch of 8 groups: transposed fp16 dma_gather pulls the
        256 negative rows/group as [h on partitions, j on free]
      * DVE tensor_scalar (4x fp16) multiplies by predicts[h] per group
      * PE "select-column" matmul (ones-column sliding window lhsT)
        simultaneously reduces over h (partitions) and routes group g's
        logits into psum row g%128 -> [128 groups, 256 negs] per block
      * softmax/logsumexp per block on DVE+ACT (exp with per-partition
        bias + accum_out), accumulate (lse - pos) per partition
      * final cross-partition sum via ones-matmul -> [1,1] partial
  - Host: loss = sum(partials) / 4096.
"""
import sys

if "/opt/trn_rl_repo" not in sys.path:
    sys.path.insert(0, "/opt/trn_rl_repo")

from contextlib import ExitStack

import numpy as np

import concourse.bass as bass
import concourse.bacc as bacc
import concourse.mybir as mybir
import concourse.tile as tile
from concourse.bass_utils import run_bass_kernel_spmd

# problem constants (hardcoded per harness contract)
N_GROUPS, K_POS, HID, M_NEG = 4096, 4, 256, 256
N_CORES = 8
GROUPS_PER_CALL = 8  # groups per dma_gather call (production setting)
TABLE_ROWS = N_GROUPS * K_POS          # 16384
CTX = (K_POS - 1) * HID                # 768
KC = CTX // 128                        # 6 contraction chunks
HC = HID // 128                        # 2 h chunks

F32 = mybir.dt.float32
F16 = mybir.dt.float16
I16 = mybir.dt.int16


def build_program(gpc: int, groups_per_call: int = 8, debug_stage: int = 5,
                  repeat: int = 1, sbuf_table: bool = False):
    """Build the per-core Tile program. gpc = groups per core.

    debug_stage (bisect aid): 1=predicts only, 2=+pos, 3=+gather/mul,
    4=+neg matmuls, 5=full softmax (production).
    repeat: run the negatives pipeline N times (timing instrumentation;
    results only valid for repeat=1).
    sbuf_table: keep the fp16 table resident in SBUF and gather from it
    (HBM sees the f32 table exactly once)."""
    assert gpc % 128 == 0 or gpc in (8, 16, 32, 64)
    nblocks = max(1, gpc // 128)
    block_sz = min(gpc, 128)
    ncalls = gpc // groups_per_call
    nidx = groups_per_call * M_NEG     # indices per gather call
    idx_cols_per_call = nidx // 16

    nc = bacc.Bacc("TRN2", target_bir_lowering=False, debug=False)

    emb = nc.dram_tensor("emb", [TABLE_ROWS, HID], F32, kind="ExternalInput")
    histxT = nc.dram_tensor("histxT", [CTX, gpc], F32, kind="ExternalInput")
    histy = nc.dram_tensor("histy", [gpc, HID], F32, kind="ExternalInput")
    wt = nc.dram_tensor("wt", [CTX, HID], F32, kind="ExternalInput")
    b_colT = nc.dram_tensor("b_colT", [128, HC], F32, kind="ExternalInput")
    b_bcast = nc.dram_tensor("b_bcast", [128, HID], F32, kind="ExternalInput")
    idx = nc.dram_tensor("idx", [128, ncalls * idx_cols_per_call], I16,
                         kind="ExternalInput")
    loss_out = nc.dram_tensor("loss", [1, 1], F32, kind="ExternalOutput")

    table16 = None if sbuf_table else nc.dram_tensor("table16", [TABLE_ROWS, HID], F16)

    with tile.TileContext(nc) as tc, ExitStack() as ctx:
        const_pool = ctx.enter_context(tc.tile_pool(name="const", bufs=1))
        gpool = ctx.enter_context(tc.tile_pool(name="gather", bufs=6))
        ppool = ctx.enter_context(tc.tile_pool(name="prod", bufs=12))
        spool = ctx.enter_context(tc.tile_pool(name="small", bufs=3))
        psum_neg = ctx.enter_context(tc.tile_pool(name="psn", bufs=3, space="PSUM"))
        psum_misc = ctx.enter_context(tc.tile_pool(name="psm", bufs=2, space="PSUM"))

        # --- fp16 table cast (SWDGE cast-DMA) ---
        if sbuf_table:
            # Partition-major layout: t16_sb[p, s, :] = emb[p*128 + s].
            # Each partition reads 128 consecutive rows (128 KB contiguous)
            # -> ~1 descriptor per partition instead of 1 per row. The host
            # compensates by permuting gather indices: token t = pi(v) =
            # (v % 128)*128 + v//128, so the gather (t%128 -> partition,
            # t//128 -> stripe) lands on emb[v].
            t16_sb = const_pool.tile([128, TABLE_ROWS // 128, HID], F16)
            nc.gpsimd.dma_start(
                t16_sb[:], emb.ap().rearrange("(p s) h -> p s h", p=128))
        else:
            nc.gpsimd.dma_start(table16.ap(), emb.ap())

        # --- constant / input loads ---
        idx_sb = const_pool.tile([128, ncalls * idx_cols_per_call], I16)
        nc.sync.dma_start(idx_sb[:], idx.ap())
        wt_sb = const_pool.tile([128, KC, HID], F32)
        nc.sync.dma_start(wt_sb[:], wt.ap().rearrange("(kc p) h -> p kc h", p=128))
        hx_sb = const_pool.tile([128, KC, gpc], F32)
        nc.sync.dma_start(hx_sb[:], histxT.ap().rearrange("(kc p) g -> p kc g", p=128))
        hy_sb = const_pool.tile([128, nblocks, HID], F32)
        nc.sync.dma_start(
            hy_sb[:block_sz, :, :],
            histy.ap().rearrange("(nb p) h -> p nb h", p=block_sz),
        )
        bcol_sb = const_pool.tile([128, HC], F32)
        nc.sync.dma_start(bcol_sb[:], b_colT.ap())
        bbc_sb = const_pool.tile([128, HID], F32)
        nc.sync.dma_start(bbc_sb[:], b_bcast.ap())

        # select matrix: zeros except col 127 = 1 (fp16)
        big = const_pool.tile([128, 256], F16)
        nc.vector.memset(big[:], 0.0)
        nc.vector.memset(big[:, 127:128], 1.0)
        ones_f32 = const_pool.tile([128, 1], F32)
        nc.vector.memset(ones_f32[:], 1.0)

        # --- predicts, orientation [h, g] (predT) ---
        predT = const_pool.tile([128, HC, gpc], F32)
        for hcx in range(HC):
            ps = psum_misc.tile([128, gpc], F32)
            for kcx in range(KC):
                nc.tensor.matmul(
                    ps[:],
                    wt_sb[:, kcx, hcx * 128:(hcx + 1) * 128],
                    hx_sb[:, kcx, :],
                    start=(kcx == 0), stop=(kcx == KC - 1),
                )
            nc.vector.tensor_scalar_add(predT[:, hcx, :], ps[:], bcol_sb[:, hcx:hcx + 1])

        # --- predicts, orientation [g, h] + positive logits ---
        pos_all = const_pool.tile([128, nblocks], F32)
        nc.vector.memset(pos_all[:], 0.0)
        for bx in range(nblocks if debug_stage >= 2 else 0):
            ps = psum_misc.tile([128, HID], F32)
            for kcx in range(KC):
                nc.tensor.matmul(
                    ps[:block_sz, :],
                    hx_sb[:, kcx, bx * block_sz:(bx + 1) * block_sz],
                    wt_sb[:, kcx, :],
                    start=(kcx == 0), stop=(kcx == KC - 1),
                )
            pred_b = spool.tile([128, HID], F32, tag="pred_b")
            nc.vector.tensor_add(pred_b[:block_sz, :], ps[:block_sz, :], bbc_sb[:block_sz, :])
            prodp = spool.tile([128, HID], F32, tag="prodp")
            nc.vector.tensor_mul(prodp[:block_sz, :], pred_b[:block_sz, :],
                                 hy_sb[:block_sz, bx, :])
            nc.vector.reduce_sum(pos_all[:block_sz, bx:bx + 1], prodp[:block_sz, :],
                                 axis=mybir.AxisListType.X)

        # --- negatives: gather -> scale -> select-reduce matmul ---
        acc = const_pool.tile([128, 1], F32)
        nc.vector.memset(acc[:], 0.0)
        exp_scratch = spool.tile([128, M_NEG], F32, tag="exps")

        psum_b = None
        for rep, call in [(rp, cl) for rp in range(repeat)
                          for cl in range(ncalls if debug_stage >= 3 else 0)]:
            gt = gpool.tile([128, HC, nidx], F16, tag="gt")
            idx_ap = idx_sb[:, call * idx_cols_per_call:(call + 1) * idx_cols_per_call]
            if sbuf_table:
                nc.gpsimd.dma_gather(
                    gt[:], t16_sb[:], idx_ap,
                    nidx, nidx, HID, transpose=True, single_packet=False,
                    sbuf_tokens_per_rank=128,
                    sbuf_free_dim_per_rank=HID * 2,
                )
            else:
                nc.gpsimd.dma_gather(
                    gt[:], table16.ap(), idx_ap,
                    nidx, nidx, HID, transpose=True, single_packet=False,
                )
            for g8 in range(groups_per_call):
                g = call * groups_per_call + g8
                bx, r = divmod(g, block_sz)
                if r == 0:
                    psum_b = psum_neg.tile([128, HC, M_NEG], F32, tag="psb")
                prod = ppool.tile([128, HC, M_NEG], F16, tag="prod")
                for hcx in range(HC):
                    nc.vector.tensor_scalar_mul(
                        prod[:, hcx, :],
                        gt[:, hcx, g8 * M_NEG:(g8 + 1) * M_NEG],
                        predT[:, hcx, g:g + 1],
                    )
                if debug_stage < 4:
                    continue
                nc.tensor.matmul(
                    psum_b[:block_sz, :, :],
                    big[:, 127 - r:127 - r + block_sz],
                    prod[:, :, :],
                    start=(r == 0), stop=(r == block_sz - 1),
                )
                if r == block_sz - 1 and debug_stage >= 5 and rep == 0:
                    # --- combine the two h-chunk partial sums -> full logits ---
                    negs_sb = spool.tile([128, M_NEG], F32, tag="negs")
                    nc.scalar.copy(negs_sb[:block_sz, :], psum_b[:block_sz, 0, :])
                    nc.vector.tensor_add(negs_sb[:block_sz, :], negs_sb[:block_sz, :],
                                         psum_b[:block_sz, 1, :])
                    # --- block softmax / logsumexp ---
                    mx = spool.tile([128, 1], F32, tag="mx")
                    nc.vector.reduce_max(mx[:block_sz, :], negs_sb[:block_sz, :],
                                         axis=mybir.AxisListType.X)
                    mx2 = spool.tile([128, 1], F32, tag="mx2")
                    nc.vector.tensor_max(mx2[:block_sz, :], mx[:block_sz, :],
                                         pos_all[:block_sz, bx:bx + 1])
                    nmx2 = spool.tile([128, 1], F32, tag="nmx2")
                    nc.vector.tensor_scalar_mul(nmx2[:block_sz, :], mx2[:block_sz, :], -1.0)
                    sumexp = spool.tile([128, 1], F32, tag="sumexp")
                    nc.scalar.activation(
                        exp_scratch[:block_sz, :], negs_sb[:block_sz, :],
                        mybir.ActivationFunctionType.Exp,
                        bias=nmx2[:block_sz, :], scale=1.0,
                        accum_out=sumexp[:block_sz, :],
                    )
                    expp = spool.tile([128, 1], F32, tag="expp")
                    nc.scalar.activation(
                        expp[:block_sz, :], pos_all[:block_sz, bx:bx + 1],
                        mybir.ActivationFunctionType.Exp,
                        bias=nmx2[:block_sz, :], scale=1.0,
                    )
                    denom = spool.tile([128, 1], F32, tag="denom")
                    nc.vector.tensor_add(denom[:block_sz, :], sumexp[:block_sz, :],
                                         expp[:block_sz, :])
                    logd = spool.tile([128, 1], F32, tag="logd")
                    nc.scalar.activation(logd[:block_sz, :], denom[:block_sz, :],
                                         mybir.ActivationFunctionType.Ln)
                    lse = spool.tile([128, 1], F32, tag="lse")
                    nc.vector.tensor_add(lse[:block_sz, :], logd[:block_sz, :],
                                         mx2[:block_sz, :])
                    li = spool.tile([128, 1], F32, tag="li")
                    nc.vector.tensor_sub(li[:block_sz, :], lse[:block_sz, :],
                                         pos_all[:block_sz, bx:bx + 1])
                    nc.vector.tensor_add(acc[:block_sz, :], acc[:block_sz, :],
                                         li[:block_sz, :])

        # --- cross-partition sum -> [1,1] ---
        ps_fin = psum_misc.tile([1, 1], F32)
        nc.tensor.matmul(ps_fin[:], ones_f32[:block_sz, :],
                         acc[:block_sz, :], start=True, stop=True)
        out_sb = spool.tile([1, 1], F32, tag="out")
        nc.vector.tensor_copy(out_sb[:], ps_fin[:])
        nc.sync.dma_start(loss_out.ap(), out_sb[:])

    nc.compile()
    return nc


def prep_core_inputs(embeddings, W, b, neg_perm, core, gpc, groups_per_call=8,
                     permute_idx=True):
    """Host-side layout prep for one core's in_map.

    permute_idx must match the program's sbuf_table setting (True for the
    production partition-major SBUF table layout)."""
    n = N_GROUPS
    k = K_POS
    g0 = core * gpc
    e = embeddings.reshape(n, k, HID)
    hist_x = e[g0:g0 + gpc, :k - 1, :].reshape(gpc, CTX)
    histxT = np.ascontiguousarray(hist_x.T)
    histy = np.ascontiguousarray(e[g0:g0 + gpc, k - 1, :])
    wt = np.ascontiguousarray(W.T)
    b_colT = np.ascontiguousarray(b.reshape(HC, 128).T)
    b_bcast = np.ascontiguousarray(np.broadcast_to(b, (128, HID)))

    gi = np.arange(g0, g0 + gpc, dtype=np.int64)[:, None]
    np_perm = neg_perm[g0:g0 + gpc].astype(np.int64)
    neg_idx = np_perm + np.where(np_perm >= gi * k, k, 0)
    assert neg_idx.max() < TABLE_ROWS
    if permute_idx:
        # token id for the partition-major SBUF table (see build_program)
        neg_idx = (neg_idx % 128) * 128 + neg_idx // 128
    neg_idx = neg_idx.astype(np.int16)

    ncalls = gpc // groups_per_call
    nidx = groups_per_call * M_NEG
    # per call: seq [nidx] -> wrapped [16, nidx/16] (g -> (g%16, g//16)),
    # replicated 8x across 128 partitions
    seq = neg_idx.reshape(ncalls, nidx)
    wrapped = seq.reshape(ncalls, nidx // 16, 16).transpose(0, 2, 1)  # [ncalls, 16, nidx/16]
    rep = np.tile(wrapped, (1, 8, 1))                                 # [ncalls, 128, nidx/16]
    idx_all = np.ascontiguousarray(rep.transpose(1, 0, 2).reshape(128, ncalls * (nidx // 16)))

    return {
        "emb": np.ascontiguousarray(embeddings, dtype=np.float32),
        "histxT": histxT.astype(np.float32),
        "histy": histy.astype(np.float32),
        "wt": wt.astype(np.float32),
        "b_colT": b_colT.astype(np.float32),
        "b_bcast": b_bcast.astype(np.float32),
        "idx": idx_all,
    }


_PROGRAM_CACHE = {}


def _get_program(gpc):
    if gpc not in _PROGRAM_CACHE:
        _PROGRAM_CACHE[gpc] = build_program(
            gpc, groups_per_call=GROUPS_PER_CALL, sbuf_table=True)
    return _PROGRAM_CACHE[gpc]


def kernel(embeddings, W, b, target, neg_perm, k_pos_samples):
    embeddings = np.asarray(embeddings, dtype=np.float32)
    W = np.asarray(W, dtype=np.float32)
    b = np.asarray(b, dtype=np.float32)
    neg_perm = np.asarray(neg_perm)
    assert int(k_pos_samples) == K_POS
    assert embeddings.shape == (TABLE_ROWS, HID)

    gpc = N_GROUPS // N_CORES
    nc = _get_program(gpc)
    in_maps = [
        prep_core_inputs(embeddings, W, b, neg_perm, core, gpc, GROUPS_PER_CALL)
        for core in range(N_CORES)
    ]
    res = run_bass_kernel_spmd(nc, in_maps, list(range(N_CORES)))
    total = sum(float(r["loss"][0, 0]) for r in res.results)
    return np.float32(total / N_GROUPS)



# revision 14
# speedup vs baseline: 1.8038x; 1.8038x over previous
"""CPC loss (nn_CPCLossV2) Trainium2 kernel — dense-logits formulation.

Strategy (data-parallel over the n=4096 group axis, 512 groups/core x 8 cores):
  Instead of gathering 256 negative rows per group (the baseline's DMA
  bottleneck: 131072 row-gathers/core), compute ALL logits densely and
  fold the data-dependent negative selection into a count matrix:

      S_g = sum_t count[g,t] * exp(logit[g,t] - K) + exp(pos_g - K)
      loss_g = ln(S_g) + K - pos_g

  where count[g,t] = |{j : neg_idx[g,j] = t}| is pure index arithmetic
  (host-prepped, uploaded as ln(count) in fp8: 0 stays 0, count=0 maps
  to -240 so exp underflows to exactly 0; no per-element gather at all).

  Device, per core (512 groups = 4 blocks x 128):
    * cast-DMA the f32 embedding table to an SBUF bf16 embT [h, t]
    * predT = W @ hist_x^T + b via PE (bf16), orientation [h, g]
    * main loop over 4 blocks x 32 chunks of 512 t-columns:
        PE: 2 bf16 matmuls -> psum [128g, 512t]
        DVE: scalar_tensor_tensor: x = psum + lnc8 (f32, 4-chunk batches)
        ACT: exp(x - K) with accum_out -> per-8-chunk partial sums
    * pos_g via [g, h]-oriented matmuls + multiply/reduce, exp(pos - K)
    * S -> Ln -> loss per group, cross-partition ones-matmul -> [1,1]
  Host: loss = sum(core partials) / 4096.
"""
import sys

if "/opt/trn_rl_repo" not in sys.path:
    sys.path.insert(0, "/opt/trn_rl_repo")

from contextlib import ExitStack

import numpy as np
import ml_dtypes

import concourse.bass as bass
import concourse.bacc as bacc
import concourse.mybir as mybir
import concourse.tile as tile
from concourse.bass_utils import run_bass_kernel_spmd

# problem constants (hardcoded per harness contract)
N_GROUPS, K_POS, HID, M_NEG = 4096, 4, 256, 256
N_CORES = 8
TABLE_ROWS = N_GROUPS * K_POS          # 16384
CTX = (K_POS - 1) * HID                # 768
KC = CTX // 128                        # 6 contraction chunks
HC = HID // 128                        # 2 h chunks

F32 = mybir.dt.float32
BF16 = mybir.dt.bfloat16
FP8 = mybir.dt.float8e4
AL = mybir.AluOpType
AF = mybir.ActivationFunctionType

K_SHIFT = 24.0          # exp shift: logits ~ N(0,16^2), global max ~ +100
LNC_ZERO = -240.0       # fp8e4 most-negative finite; exp(x-240-K) == 0 in f32

NBLK = 4                # g-blocks of 128 per core
TCH = 32                # t-chunks of 512 per block
CHUNK = 512
STT_B = 4               # chunks per DVE scalar_tensor_tensor (one 4-bank psum tile)
ACT_B = 8               # chunks per ACT exp instruction (2 stt batches)


def build_program(gpc: int, dbg: bool = False):
    assert gpc == N_GROUPS // N_CORES == NBLK * 128
    nc = bacc.Bacc("TRN2", target_bir_lowering=False, debug=False)

    embT = nc.dram_tensor("embT", [HID, TABLE_ROWS], F32, kind="ExternalInput")
    histxT = nc.dram_tensor("histxT", [CTX, gpc], F32, kind="ExternalInput")
    histy = nc.dram_tensor("histy", [gpc, HID], F32, kind="ExternalInput")
    wt = nc.dram_tensor("wt", [CTX, HID], F32, kind="ExternalInput")
    b_colT = nc.dram_tensor("b_colT", [128, HC], F32, kind="ExternalInput")
    b_row = nc.dram_tensor("b_row", [1, HID], F32, kind="ExternalInput")
    lnc = nc.dram_tensor("lnc", [128, NBLK * TCH * CHUNK], FP8,
                         kind="ExternalInput")
    loss_out = nc.dram_tensor("loss", [1, 1], F32, kind="ExternalOutput")
    if dbg:
        dbg_pos = nc.dram_tensor("dbg_pos", [128, NBLK], F32, kind="ExternalOutput")
        dbg_s = nc.dram_tensor("dbg_s", [128, NBLK * 5], F32, kind="ExternalOutput")
        dbg_x = nc.dram_tensor("dbg_x", [128, ACT_B * CHUNK], F32, kind="ExternalOutput")
        dbg_ps = nc.dram_tensor("dbg_ps", [128, CHUNK], F32, kind="ExternalOutput")
        dbg_fin = nc.dram_tensor("dbg_fin", [128, 3 * NBLK + 1], F32, kind="ExternalOutput")

    with tile.TileContext(nc) as tc, ExitStack() as ctx:
        cpool = ctx.enter_context(tc.tile_pool(name="const", bufs=1))
        xpool = ctx.enter_context(tc.tile_pool(name="x", bufs=3))
        epool = ctx.enter_context(tc.tile_pool(name="escr", bufs=1))
        spool = ctx.enter_context(tc.tile_pool(name="small", bufs=2))
        ppool = ctx.enter_context(tc.tile_pool(name="psA", bufs=2, space="PSUM"))

        # ---- constant / input loads ----
        emb16 = cpool.tile([128, HC, TABLE_ROWS], BF16)
        nc.gpsimd.dma_start(emb16[:], embT.ap().rearrange("(c p) t -> p c t", p=128))
        lnc_sb = cpool.tile([128, NBLK, TCH, CHUNK], FP8)
        nc.sync.dma_start(
            lnc_sb[:], lnc.ap().rearrange("p (b c j) -> p b c j", b=NBLK, c=TCH))
        hx16 = cpool.tile([128, KC, gpc], BF16)
        nc.gpsimd.dma_start(hx16[:], histxT.ap().rearrange("(k p) g -> p k g", p=128))
        wt16 = cpool.tile([128, KC, HID], BF16)
        nc.gpsimd.dma_start(wt16[:], wt.ap().rearrange("(k p) h -> p k h", p=128))
        hy16 = cpool.tile([128, NBLK, HID], BF16)
        nc.gpsimd.dma_start(hy16[:], histy.ap().rearrange("(b p) h -> p b h", p=128))
        brow16 = cpool.tile([1, HID], BF16)
        nc.gpsimd.dma_start(brow16[:], b_row.ap())
        bcol_sb = cpool.tile([128, HC], F32)
        nc.sync.dma_start(bcol_sb[:], b_colT.ap())

        negk = cpool.tile([128, 1], F32)
        nc.vector.memset(negk[:], -K_SHIFT)
        ones1 = cpool.tile([1, 128], BF16)
        nc.vector.memset(ones1[:], 1.0)
        ones_f32 = cpool.tile([128, 1], F32)
        nc.vector.memset(ones_f32[:], 1.0)

        # ---- predT [h, g] = (W @ hist_x^T + b), bf16 ----
        predT16 = cpool.tile([128, HC, gpc], BF16)
        for hcx in range(HC):
            pst0 = ppool.tile([128, STT_B, CHUNK], F32, tag="pst")
            ps = pst0[:, 0, :]
            for kcx in range(KC):
                nc.tensor.matmul(
                    ps, wt16[:, kcx, hcx * 128:(hcx + 1) * 128],
                    hx16[:, kcx, :], start=(kcx == 0), stop=(kcx == KC - 1))
            nc.scalar.activation(predT16[:, hcx, :], ps, AF.Identity,
                                 bias=bcol_sb[:, hcx:hcx + 1], scale=1.0)

        # ---- pos path: predicts [g, h] per block + dot(hist_y) ----
        pos_col = cpool.tile([128, NBLK], F32)
        prod_scr = spool.tile([128, HID], F32, tag="prod")
        for bx in range(NBLK):
            pst0 = ppool.tile([128, STT_B, CHUNK], F32, tag="pst")
            psp = pst0[:, 0, 0:HID]
            nc.tensor.matmul(psp, ones1[:], brow16[:], start=True, stop=False)
            for kcx in range(KC):
                nc.tensor.matmul(
                    psp, hx16[:, kcx, bx * 128:(bx + 1) * 128],
                    wt16[:, kcx, :], start=False, stop=(kcx == KC - 1))
            nc.vector.scalar_tensor_tensor(prod_scr[:], psp, 1.0,
                                           hy16[:, bx, :], AL.mult, AL.mult)
            nc.vector.tensor_reduce(pos_col[:, bx:bx + 1], prod_scr[:],
                                    mybir.AxisListType.X, AL.add)

        # s_cols layout [128, NBLK*5]: per block, 4 ACT partial sums + exp(pos-K)
        s_cols = cpool.tile([128, NBLK * 5], F32)
        for bx in range(NBLK):
            nc.scalar.activation(s_cols[:, bx * 5 + 4:bx * 5 + 5],
                                 pos_col[:, bx:bx + 1], AF.Exp,
                                 bias=negk[:], scale=1.0)

        # ---- main loop: dense logits -> +lnc -> exp-accum ----
        for bx in range(NBLK):
            for ag in range(TCH // ACT_B):          # 4 ACT groups of 8 chunks
                x_sb = xpool.tile([128, ACT_B, CHUNK], F32, tag="x")
                for sg in range(ACT_B // STT_B):    # 2 stt batches of 4 chunks
                    pst = ppool.tile([128, STT_B, CHUNK], F32, tag="pst")
                    for cc in range(STT_B):
                        c = ag * ACT_B + sg * STT_B + cc
                        for hcx in range(HC):
                            nc.tensor.matmul(
                                pst[:, cc, :],
                                predT16[:, hcx, bx * 128:(bx + 1) * 128],
                                emb16[:, hcx, c * CHUNK:(c + 1) * CHUNK],
                                start=(hcx == 0), stop=(hcx == HC - 1))
                    c0 = ag * ACT_B + sg * STT_B
                    if dbg and bx == 0 and ag == 0 and sg == 0:
                        psc = spool.tile([128, CHUNK], F32, tag="dps")
                        nc.vector.tensor_copy(psc[:], pst[:, 0, :])
                        nc.sync.dma_start(dbg_ps.ap(), psc[:])
                    nc.vector.scalar_tensor_tensor(
                        x_sb[:, sg * STT_B:(sg + 1) * STT_B, :], pst[:], 1.0,
                        lnc_sb[:, bx, c0:c0 + STT_B, :], AL.mult, AL.add)
                e_scr = epool.tile([128, ACT_B, CHUNK], BF16, tag="e")
                nc.scalar.activation(e_scr[:], x_sb[:], AF.Exp, bias=negk[:],
                                     scale=1.0,
                                     accum_out=s_cols[:, bx * 5 + ag:bx * 5 + ag + 1])
                if dbg and bx == 0 and ag == 0:
                    nc.sync.dma_start(
                        dbg_x.ap().rearrange("p (c j) -> p c j", c=ACT_B), x_sb[:])

        # ---- finalize: S -> ln -> loss -> cross-partition sum ----
        s_red = spool.tile([128, NBLK], F32, tag="sred")
        nc.vector.tensor_reduce(
            s_red[:], s_cols[:].rearrange("p (b f) -> p b f", b=NBLK),
            mybir.AxisListType.X, AL.add)
        # HW ACT Ln is wrong for large inputs -> frexp split:
        # ln(S) = Ln(mantissa in [1,2)) + exponent * ln2
        U32 = mybir.dt.uint32
        s_bits = s_red[:].bitcast(U32)
        e_u = spool.tile([128, NBLK], U32, tag="ef")
        nc.vector.tensor_scalar(e_u[:], s_bits, 23, None,
                                AL.logical_shift_right)
        m_bits = spool.tile([128, NBLK], U32, tag="mbits")
        nc.vector.tensor_scalar(m_bits[:], s_bits, 0x007FFFFF, 0x3F800000,
                                AL.bitwise_and, AL.bitwise_or)
        ln_m = spool.tile([128, NBLK], F32, tag="lnm")
        nc.scalar.activation(ln_m[:], m_bits[:].bitcast(F32), AF.Ln)
        ln_s = spool.tile([128, NBLK], F32, tag="lns")
        nc.vector.scalar_tensor_tensor(ln_s[:], e_u[:], 0.6931471805599453,
                                       ln_m[:], AL.mult, AL.add)
        loss_cols = spool.tile([128, NBLK], F32, tag="lcols")
        # K_SHIFT - 127*ln2 folds the frexp exponent bias into the shift
        nc.vector.scalar_tensor_tensor(loss_cols[:], ln_s[:],
                                       K_SHIFT - 127.0 * 0.6931471805599453,
                                       pos_col[:], AL.add, AL.subtract)
        loss_red = spool.tile([128, 1], F32, tag="lred")
        nc.vector.tensor_reduce(loss_red[:], loss_cols[:],
                                mybir.AxisListType.X, AL.add)
        if dbg:
            nc.sync.dma_start(dbg_pos.ap(), pos_col[:])
            nc.sync.dma_start(dbg_s.ap(), s_cols[:])
        pst0 = ppool.tile([128, STT_B, CHUNK], F32, tag="pst")
        ps_fin = pst0[0:1, 0, 0:1]
        nc.tensor.matmul(ps_fin, loss_red[:], ones_f32[:],
                         start=True, stop=True)
        out_sb = spool.tile([1, 1], F32, tag="out")
        nc.vector.tensor_copy(out_sb[:], ps_fin)
        if dbg:
            fin = spool.tile([128, 3 * NBLK + 1], F32, tag="fin")
            nc.vector.tensor_copy(fin[:, 0:NBLK], s_red[:])
            nc.vector.tensor_copy(fin[:, NBLK:2 * NBLK], ln_s[:])
            nc.vector.tensor_copy(fin[:, 2 * NBLK:3 * NBLK], loss_cols[:])
            nc.vector.tensor_copy(fin[:, 3 * NBLK:], loss_red[:])
            nc.sync.dma_start(dbg_fin.ap(), fin[:])
        nc.sync.dma_start(loss_out.ap(), out_sb[:])

    nc.compile()
    return nc


_LNC_LUT = None


def _lnc_lut():
    global _LNC_LUT
    if _LNC_LUT is None:
        vals = np.full(64, LNC_ZERO, np.float32)
        vals[1:] = np.log(np.arange(1, 64, dtype=np.float32))
        vals[0] = LNC_ZERO
        _LNC_LUT = vals.astype(ml_dtypes.float8_e4m3).view(np.uint8)
    return _LNC_LUT


def prep_core_inputs(embT_c, wt_c, b_colT_c, b_row_c, embeddings, neg_perm, core,
                     gpc):
    """Host-side layout/index prep for one core's in_map (no float math on
    reference data beyond dtype-preserving transposes/slices)."""
    n, k = N_GROUPS, K_POS
    g0 = core * gpc
    e = embeddings.reshape(n, k, HID)
    hist_x = e[g0:g0 + gpc, :k - 1, :].reshape(gpc, CTX)
    histxT = np.ascontiguousarray(hist_x.T)
    histy = np.ascontiguousarray(e[g0:g0 + gpc, k - 1, :])

    # counts over global table rows for this core's groups
    gi = np.arange(g0, g0 + gpc, dtype=np.int64)[:, None]
    np_perm = neg_perm[g0:g0 + gpc].astype(np.int64)
    neg_idx = np_perm + np.where(np_perm >= gi * k, k, 0)       # [gpc, 256]
    flat = (np.arange(gpc, dtype=np.int64)[:, None] * TABLE_ROWS + neg_idx).ravel()
    cnt = np.bincount(flat, minlength=gpc * TABLE_ROWS).astype(np.int64)
    cnt = cnt.reshape(gpc, TABLE_ROWS)
    lnc_bytes = _lnc_lut()[np.minimum(cnt, 63)]                 # [gpc, 16384] u8
    # device layout [128, NBLK, TCH, CHUNK]: partition = g % 128, b = g // 128
    lnc_dev = np.ascontiguousarray(
        lnc_bytes.reshape(NBLK, 128, TCH * CHUNK)
        .transpose(1, 0, 2).reshape(128, NBLK * TCH * CHUNK))

    return {
        "embT": embT_c,
        "histxT": histxT.astype(np.float32),
        "histy": histy.astype(np.float32),
        "wt": wt_c,
        "b_colT": b_colT_c,
        "b_row": b_row_c,
        "lnc": lnc_dev.view(ml_dtypes.float8_e4m3),
    }


_PROGRAM_CACHE = {}


def _get_program(gpc):
    if gpc not in _PROGRAM_CACHE:
        _PROGRAM_CACHE[gpc] = build_program(gpc)
    return _PROGRAM_CACHE[gpc]


def kernel(embeddings, W, b, target, neg_perm, k_pos_samples):
    embeddings = np.asarray(embeddings, dtype=np.float32)
    W = np.asarray(W, dtype=np.float32)
    b = np.asarray(b, dtype=np.float32)
    neg_perm = np.asarray(neg_perm)
    assert int(k_pos_samples) == K_POS
    assert embeddings.shape == (TABLE_ROWS, HID)

    gpc = N_GROUPS // N_CORES
    nc = _get_program(gpc)

    embT_c = np.ascontiguousarray(embeddings.T)                 # [256, 16384]
    wt_c = np.ascontiguousarray(W.T).astype(np.float32)         # [768, 256]
    b_colT_c = np.ascontiguousarray(b.reshape(HC, 128).T).astype(np.float32)
    b_row_c = np.ascontiguousarray(b.reshape(1, HID)).astype(np.float32)

    in_maps = [
        prep_core_inputs(embT_c, wt_c, b_colT_c, b_row_c, embeddings, neg_perm,
                         core, gpc)
        for core in range(N_CORES)
    ]
    res = run_bass_kernel_spmd(nc, in_maps, list(range(N_CORES)))
    total = sum(float(r["loss"][0, 0]) for r in res.results)
    return np.float32(total / N_GROUPS)
